# revision 26
# baseline (speedup 1.0000x reference)
"""MoE (top-2 of 16 experts, SwiGLU MLP) kernel for 8 Trainium2 NeuronCores.

Strategy (expert-parallel, per sharding hint):
  - Host: router (x @ w_gate -> softmax -> top-2) in float64; tokens
    gathered per expert. Experts ranked by token count: 8 largest ->
    core slot 0, 8 smallest -> slot 1; each slot gets a uniform
    capacity = its max count rounded to 16 (exact, no 128-rounding).
  - Everything streamed to the device is bf16 and pre-laid-out on host
    in the exact SBUF tile layout, so every DMA descriptor moves a
    contiguous multi-KB run (~45 DMAs / ~11K descriptors per core vs
    62K for naive row-wise transfers). All weights + tokens fit in
    SBUF at bf16, so the whole input set is prefetched up front and
    the PE never waits on a mid-kernel weight load.
  - Device (SPMD over 8 cores, 2 experts/core): per expert
        ht = silu(W1e.T @ Xt) * (W2e.T @ Xt)     [feature-major]
        yt = WCe.T @ ht                           (gate applied on host)
    bf16 matmuls accumulate fp32 in PSUM (same PE rate as float32r at
    free-dim >= 256, but half the DMA/LDWEIGHTS traffic).
  - Host: out[tok] += gate * yt  (scatter-add, fp32).
"""

import contextlib
import ctypes
import os
import sys
import types

sys.path.insert(0, "/opt/trn_rl_repo")

import ml_dtypes
import numpy as np

import concourse.bass as bass
import concourse.mybir as mybir
import concourse.tile as tile

EMB = 1024
HID = 1024
E = 16
TOPK = 2
NCORES = 8
EPC = E // NCORES  # experts per core
P = 128
KT = EMB // P  # contraction tiles (8)
HT = HID // P  # hidden row-blocks (8)
BF16 = ml_dtypes.bfloat16


def _install_profile_shim():
    """Register the axon NTFF profiling hook (missing antenv.axon_hooks in
    this image) so run_bass_kernel_spmd(trace=True) can measure HW time."""
    if "antenv.axon_hooks" in sys.modules:
        return
    try:
        lib = ctypes.CDLL("/opt/axon/libaxon_pjrt.so")
        lib.axon_start_nrt_profile.argtypes = [
            ctypes.POINTER(ctypes.c_int64),
            ctypes.c_size_t,
        ]
        lib.axon_start_nrt_profile.restype = ctypes.c_int64
        lib.axon_stop_nrt_profile.argtypes = [ctypes.c_char_p]
        lib.axon_stop_nrt_profile.restype = ctypes.c_int64
    except Exception:
        return

    @contextlib.contextmanager
    def _hook(output_dir, device_ids):
        import jax

        jax.devices()
        ids = (
            (ctypes.c_int64 * len(device_ids))(*device_ids) if device_ids else None
        )
        rc = lib.axon_start_nrt_profile(ids, len(device_ids) if device_ids else 0)
        if rc != 0:
            raise RuntimeError(f"axon_start_nrt_profile rc={rc}")
        try:
            yield
        finally:
            n = lib.axon_stop_nrt_profile(str(output_dir).encode())
            print(f"profile: {n} file(s) written to {output_dir}")

    mod = types.ModuleType("antenv.axon_hooks")
    mod.get_axon_ntff_profile_hook = lambda: _hook
    mod.set_axon_ntff_profile_hook = lambda h: None
    sys.modules["antenv.axon_hooks"] = mod


def _split_multi_waits(nc):
    """This container's walrus only encodes one sem wait per CTRL-class
    instruction; hoist extra waits onto dedicated single-wait NoOps."""
    idx = 0
    for fn in nc.m.functions:
        for bb in fn.blocks:
            new = []
            for inst in bb.instructions:
                si = inst.sync_info
                if si is not None and len(si.on_wait) > 1:
                    waits = list(si.on_wait)
                    for w in waits[:-1]:
                        c = mybir.InstNoOp(name=f"wsplit-{idx}", ins=[], outs=[])
                        idx += 1
                        c.engine = inst.engine
                        c.sync_info = mybir.SyncInfo(on_wait=[w], on_update=[])
                        new.append(c)
                    si.on_wait = [waits[-1]]
                new.append(inst)
            bb.instructions = new


def _token_chunks(C):
    """Split C into chunks <=512, remainder first (the remainder chunk is
    the first thing the PE touches, so keep it small for a fast start)."""
    rem = C % 512
    return ([rem] if rem else []) + [512] * (C // 512)


def _build_bass(caps):
    F32 = mybir.dt.float32
    BF = mybir.dt.bfloat16

    nc = bass.Bass()
    chunk_lists = [_token_chunks(caps[0]), _token_chunks(caps[1])]
    xt_d, w1_d, w2_d, wc_d, yt_d = [], [], [], [], []
    for e in range(EPC):
        C = caps[e]
        # all pre-laid-out on host: per-partition rows are contiguous.
        # xt is chunk-major: [chunk0: k0..k7 x cs0][chunk1: ...] so each
        # chunk is ONE dispatch (DIRECT2D dispatch costs ~600ns serial).
        xt_d.append(nc.declare_dram_parameter(f"xt{e}", [P, KT * C], BF, isOutput=False))
        w1_d.append(nc.declare_dram_parameter(f"w1_{e}", [P, HT, KT, P], BF, isOutput=False))
        w2_d.append(nc.declare_dram_parameter(f"w2_{e}", [P, HT, KT, P], BF, isOutput=False))
        wc_d.append(nc.declare_dram_parameter(f"wc_{e}", [P, HT, KT, P], BF, isOutput=False))
        yt_d.append(nc.declare_dram_parameter(f"yt{e}", [P, HT, C], BF, isOutput=True))

    with tile.TileContext(nc) as tc:
        with (
            tc.tile_pool(name="xt", bufs=1) as xt_pool,
            tc.tile_pool(name="ht", bufs=1) as ht_pool,
            tc.tile_pool(name="w", bufs=1) as w_pool,
            tc.tile_pool(name="s", bufs=3) as s_pool,
            tc.tile_pool(name="y", bufs=2) as y_pool,
            tc.tile_pool(name="psA", bufs=2, space="PSUM") as psA,
            tc.tile_pool(name="psB", bufs=3, space="PSUM") as psB,
            tc.tile_pool(name="psF", bufs=1, space="PSUM") as psF,
        ):
            # --- long-lived SBUF tiles (everything fits at bf16) ---
            xt_sb = [
                xt_pool.tile([P, KT * caps[e]], BF, tag=f"xt{e}", name=f"xt_sb{e}")
                for e in range(EPC)
            ]
            ht_sb = [
                ht_pool.tile([P, HT, caps[e]], BF, tag=f"ht{e}", name=f"ht_sb{e}")
                for e in range(EPC)
            ]
            w1_sb = [
                w_pool.tile([P, HT, KT, P], BF, tag=f"w1{e}", name=f"w1_sb{e}")
                for e in range(EPC)
            ]
            w2_sb = [
                w_pool.tile([P, HT, KT, P], BF, tag=f"w2{e}", name=f"w2_sb{e}")
                for e in range(EPC)
            ]
            wc_sb = [
                w_pool.tile([P, HT, KT, P], BF, tag=f"wc{e}", name=f"wc_sb{e}")
                for e in range(EPC)
            ]

            # --- input DMA, in consumption order. The shared HW queues
            # serve transfers in dispatch-completion order, so the order
            # below IS the arrival order: c0, w1h0, w2h0, then the 512
            # chunks in 2-k-wide pieces that land just ahead of the PE's
            # k-loop. h0 weights dispatch on the Activation HWDGE engine
            # (a second ~600ns/dispatch stream parallel to SP's). ---
            def xt_dma(e, off, cs, k0=0, k1=KT):
                nc.sync.dma_start(
                    xt_sb[e][:, off * KT + k0 * cs : off * KT + k1 * cs],
                    xt_d[e][:, off * KT + k0 * cs : off * KT + k1 * cs],
                )

            nc.scalar.dma_start(w1_sb[0][:, 0], w1_d[0][:, 0])
            nc.scalar.dma_start(w2_sb[0][:, 0], w2_d[0][:, 0])
            c0 = 0
            for ci, cs in enumerate(chunk_lists[0]):
                if ci == 0:
                    xt_dma(0, c0, cs)
                else:
                    for k in range(0, KT, 2):
                        xt_dma(0, c0, cs, k, k + 2)
                c0 += cs
            for h in range(1, HT):
                nc.sync.dma_start(w1_sb[0][:, h], w1_d[0][:, h])
                nc.sync.dma_start(w2_sb[0][:, h], w2_d[0][:, h])
            nc.sync.dma_start(wc_sb[0][:], wc_d[0][:])
            c0 = 0
            for cs in chunk_lists[1]:
                xt_dma(1, c0, cs)
                c0 += cs
            nc.sync.dma_start(w1_sb[1][:], w1_d[1][:])
            nc.sync.dma_start(w2_sb[1][:], w2_d[1][:])
            nc.sync.dma_start(wc_sb[1][:], wc_d[1][:])

            def xt_mv(e, off, k, cs):
                # moving operand: [128, cs] slice of chunk at offset `off`
                base = off * KT + k * cs
                return xt_sb[e][:, base : base + cs]

            # --- HAM warm-up on a never-written scratch tile: zero input
            # dependencies, so these run the moment the tensor engine
            # boots (~2us before the first data lands) and the clock ramp
            # completes before real work starts. Lifting the core clock
            # also doubles the DMA rate, so the head transfers land fast.
            # Garbage values land in the never-read psF bank. ---
            warm_sb = s_pool.tile([P, 512], BF, tag="warm")
            nc.vector.memset(warm_sb[:], 0.0)
            warm_ps = psF.tile([P, 512], F32, tag="fill")
            for _ in range(9):
                nc.tensor.matmul(
                    warm_ps[:], warm_sb[:, 0:P], warm_sb[:], start=True, stop=True
                )

            def a_group(e, h, c0, cs, w_sb, ps):
                for k in range(KT):
                    nc.tensor.matmul(
                        ps[:],
                        w_sb[e][:, h, k],
                        xt_mv(e, c0, k, cs),
                        start=(k == 0),
                        stop=(k == KT - 1),
                    )

            def a_evict(e, h, c0, cs, ps1, ps2):
                s_sb = s_pool.tile([P, 512], F32, tag="s")
                nc.scalar.activation(
                    s_sb[:, :cs], ps1[:], mybir.ActivationFunctionType.Silu
                )
                nc.vector.tensor_mul(
                    ht_sb[e][:, h, c0 : c0 + cs], s_sb[:, :cs], ps2[:]
                )

            for e in range(EPC):
                C = caps[e]
                chunks = chunk_lists[e]
                # Phase A: ht = silu(W1.T @ Xt) * (W2.T @ Xt)
                # e0 walks chunk-outer/h-inner: the first (small) chunk is
                # resident almost immediately and its 8 h-panels are a long
                # gap-free runway paced by the weight stream, so the HAM
                # clock lifts once, early — after which the DMA queues also
                # run at full rate and everything else lands ahead of use.
                for h in range(HT):
                    if e == 0 and h == 0 and len(chunks) >= 2:
                        # Head interleave: both w1-groups before the first
                        # w2-group, so the PE's first ~2.5us only needs
                        # w1[h0] while w2[h0] is still in flight.
                        (csa, csb), rest = chunks[:2], chunks[2:]
                        p1a = psA.tile([P, csa], F32, tag="ps1")
                        p1b = psA.tile([P, csb], F32, tag="ps1")
                        p2a = psA.tile([P, csa], F32, tag="ps2")
                        p2b = psA.tile([P, csb], F32, tag="ps2")
                        a_group(0, 0, 0, csa, w1_sb, p1a)
                        # c1's 2-k DMA pieces arrive ~0.4us apart while the
                        # PE drains each in ~0.43us: bridge the race with
                        # fillers so the HAM never sees a gap here (this
                        # bubble re-throttles the clock on ~half the runs)
                        for k in range(KT):
                            nc.tensor.matmul(
                                p1b[:, :csb],
                                w1_sb[0][:, 0, k],
                                xt_mv(0, csa, k, csb),
                                start=(k == 0),
                                stop=(k == KT - 1),
                            )
                            if k in (1, 3):
                                for _ in range(2):
                                    nc.tensor.matmul(
                                        warm_ps[:],
                                        warm_sb[:, 0:P],
                                        warm_sb[:],
                                        start=True,
                                        stop=True,
                                    )
                        a_group(0, 0, 0, csa, w2_sb, p2a)
                        a_group(0, 0, csa, csb, w2_sb, p2b)
                        a_evict(0, 0, 0, csa, p1a, p2a)
                        a_evict(0, 0, csa, csb, p1b, p2b)
                        c0 = csa + csb
                        tail = rest
                    else:
                        c0 = 0
                        tail = chunks
                    for cs in tail:
                        ps1 = psA.tile([P, cs], F32, tag="ps1")
                        ps2 = psA.tile([P, cs], F32, tag="ps2")
                        a_group(e, h, c0, cs, w1_sb, ps1)
                        a_group(e, h, c0, cs, w2_sb, ps2)
                        a_evict(e, h, c0, cs, ps1, ps2)
                        c0 += cs

                # Phase B: yt = WC.T @ ht  (gate applied on host).
                # One y store per d-row: evict chunks into a [P, C] strip,
                # single DMA dispatch when the strip completes.
                for d in range(HT):
                    y_sb = y_pool.tile([P, C], BF, tag="y")
                    c0 = 0
                    for cs in chunks:
                        psy = psB.tile([P, cs], F32, tag="psy")
                        for h in range(HT):
                            nc.tensor.matmul(
                                psy[:],
                                wc_sb[e][:, d, h],
                                ht_sb[e][:, h, c0 : c0 + cs],
                                start=(h == 0),
                                stop=(h == HT - 1),
                            )
                        nc.scalar.copy(y_sb[:, c0 : c0 + cs], psy[:])
                        c0 += cs
                    nc.sync.dma_start(yt_d[e][:, d], y_sb[:])

            # --- tail clock-hold: dependency-free matmuls keep the HAM
            # duty-cycle monitor fed while the last evict/store/teardown
            # drains, so those run at full clock. ---
            fill_ps = psF.tile([P, 512], F32, tag="fill")
            for _ in range(6):
                nc.tensor.matmul(
                    fill_ps[:],
                    w1_sb[1][:, 0, 0],
                    xt_sb[1][:, 0:512],
                    start=True,
                    stop=True,
                )

    _split_multi_waits(nc)
    return nc


def kernel(x, w_gate, w1, w2, wc):
    trace = bool(int(os.environ.get("BASS_MOE_TRACE", "0")))
    if trace:
        _install_profile_shim()

    import concourse.bass_utils as bass_utils

    bass_utils.upload_artifacts = lambda tmpdir: f"local://{tmpdir}"

    x = np.asarray(x, dtype=np.float32)
    w_gate = np.asarray(w_gate, dtype=np.float32)
    w1 = np.asarray(w1, dtype=np.float32)
    w2 = np.asarray(w2, dtype=np.float32)
    wc = np.asarray(wc, dtype=np.float32)

    b, s, d = x.shape
    xf = x.reshape(-1, d)
    n = xf.shape[0]

    # ---- Router on host (float64: stable ranking + gate values) ----
    logits = xf.astype(np.float64) @ w_gate.astype(np.float64)
    mx = logits.max(axis=1, keepdims=True)
    p = np.exp(logits - mx)
    p /= p.sum(axis=1, keepdims=True)
    top = np.argpartition(-logits, TOPK, axis=1)[:, :TOPK]  # top-2 ids (unordered)

    sel_tok = []  # per expert: token indices
    sel_gate = []  # per expert: gate values
    flat_e = top.ravel()
    flat_t = np.repeat(np.arange(n), TOPK)
    order = np.argsort(flat_e, kind="stable")
    se, st = flat_e[order], flat_t[order]
    bounds = np.searchsorted(se, np.arange(E + 1))
    counts = np.diff(bounds)
    for e in range(E):
        toks = st[bounds[e] : bounds[e + 1]]
        sel_tok.append(toks)
        sel_gate.append(p[toks, e])

    # ---- Slot assignment: biggest experts in slot 0, smallest in slot 1,
    # so each slot's uniform capacity hugs its experts' actual counts ----
    rank = np.argsort(-counts, kind="stable")
    slot_experts = [
        [int(rank[core + j * NCORES]) for j in range(EPC)] for core in range(NCORES)
    ]
    caps = []
    for j in range(EPC):
        cmax = max(counts[slot_experts[core][j]] for core in range(NCORES))
        caps.append(max(16, int(-(-cmax // 8) * 8)))

    # ---- Per-core input maps: bf16, pre-laid-out in SBUF tile order ----
    xf_bf = xf.astype(BF16)
    # weight layout [p, h, k, q] = w[k*128+p, h*128+q] (contiguous per
    # partition-row => 1 DMA descriptor per partition)
    def wlayout(w):
        return np.ascontiguousarray(
            w.astype(BF16).reshape(KT, P, HT, P).transpose(1, 2, 0, 3)
        )

    chunk_lists = [_token_chunks(caps[0]), _token_chunks(caps[1])]
    in_maps = []
    for core in range(NCORES):
        m = {}
        for j in range(EPC):
            e = slot_experts[core][j]
            C = caps[j]
            toks = sel_tok[e]
            # chunk-major: [chunk: [k: [c]]] flattened to [P, KT*C]
            xe = np.zeros((P, KT, C), dtype=BF16)
            xg = xf_bf[toks].T.reshape(KT, P, len(toks))
            xe[:, :, : len(toks)] = xg.transpose(1, 0, 2)
            xt = np.empty((P, KT * C), dtype=BF16)
            c0 = 0
            for cs in chunk_lists[j]:
                xt[:, c0 * KT : (c0 + cs) * KT] = xe[:, :, c0 : c0 + cs].reshape(
                    P, KT * cs
                )
                c0 += cs
            m[f"xt{j}"] = xt
            m[f"w1_{j}"] = wlayout(w1[e])
            m[f"w2_{j}"] = wlayout(w2[e])
            m[f"wc_{j}"] = wlayout(wc[e])
        in_maps.append(m)

    nc = _build_bass(caps)
    res = bass_utils.run_bass_kernel_spmd(
        nc, in_maps, list(range(NCORES)), trace=trace
    )
    if trace:
        kernel.last_exec_time_ns = res.exec_time_ns
        kernel.last_trace = (
            res.instructions_and_trace[1] if res.instructions_and_trace else None
        )

    # ---- Scatter-add back to token order, applying gates on host ----
    out = np.zeros((n, d), dtype=np.float64)
    for core in range(NCORES):
        for j in range(EPC):
            e = slot_experts[core][j]
            toks = sel_tok[e]
            yt = res.results[core][f"yt{j}"]  # [P, HT, C] bf16
            yv = (
                yt[:, :, : len(toks)]
                .transpose(1, 0, 2)
                .reshape(EMB, len(toks))
                .astype(np.float64)
            )
            out[toks] += sel_gate[e][:, None] * yv.T
    return out.astype(np.float32).reshape(b, s, d)
